# revision 3
# baseline (speedup 1.0000x reference)
"""AdaptiveCornerLoss on 8 TRN2 NeuronCores — batch-parallel Bass/Tile kernel.

Shapes (hardcoded): B=64, N=16384, C=6, M=128 corners. 8 cores, 8 samples/core.

Math:
  focal    = u^2 * ce  with  y=(1-2t)*x, ce=softplus(y)=ln(1+e^y),
             u=sigmoid(y)  =>  u^2 = exp(-2*ln(1+e^{-y}))
  d2(n,m)  = |p|^2 + |c|^2 - 2 p.c   (augmented K=5 fp16 matmul:
             lhsT rows [px,py,pz,|p|^2,1], rhs rows [-2cx,-2cy,-2cz,1,|c|^2+pen])
  invalid corners (-1 sentinel) get pen=+32 so they never win the min and an
  all-invalid sample yields w = exp(-sqrt(~35)*10) = 0, matching the reference.
  w        = exp(-10*sqrt(max(min_m d2, 1e-12))) via exp/ln only (one ACT set)
Outputs per core: per-partition partial sums [128,2] of (focal, focal*w);
host reduces and forms (total, focal_loss, distance_loss).
"""

import sys

sys.path.insert(0, "/opt/trn_rl_repo")

import numpy as np

import concourse.bass as bass
import concourse.mybir as mybir
from concourse import tile
from concourse.bass_utils import run_bass_kernel_spmd

NCORES = 8
B, N, M = 64, 16384, 128
S = B // NCORES          # samples per core
K = 5                    # augmented feature rows
CH = N // 128            # 128-point chunks per sample
COLS = S * CH            # minsq/logit columns per core (1024)
PEN = 32.0

F = mybir.ActivationFunctionType
OP = mybir.AluOpType
DT = mybir.dt

_CACHE = {}


def build_nc():
    nc = bass.Bass()
    lhsT = nc.declare_dram_parameter("lhsT", [S, K, N], DT.float16, isOutput=False)
    rhs = nc.declare_dram_parameter("rhs", [K, S * 128], DT.float16, isOutput=False)
    lg = nc.declare_dram_parameter("lg", [128, COLS], DT.float32, isOutput=False)
    tg = nc.declare_dram_parameter("tg", [128, COLS], DT.float32, isOutput=False)
    out = nc.declare_dram_parameter("out", [128, 2], DT.float32, isOutput=True)

    with tile.TileContext(nc) as tc:
        with (
            tc.tile_pool(name="persist", bufs=1) as pp,
            tc.tile_pool(name="stream", bufs=2) as wp,
            tc.tile_pool(name="psum", bufs=4, space="PSUM") as psp,
        ):
            # --- inputs that live on-chip for the whole kernel
            rt = pp.tile([K, S * 128], DT.float16)
            nc.sync.dma_start(out=rt[:], in_=rhs[:])
            lgt = pp.tile([128, COLS], DT.float32)
            nc.sync.dma_start(out=lgt[:], in_=lg[:])
            tgt = pp.tile([128, COLS], DT.float32)
            nc.sync.dma_start(out=tgt[:], in_=tg[:])

            sums = pp.tile([128, 2], DT.float32)

            # --- focal chain (ACT + a little DVE); overlaps the PE/DVE grid work
            y = pp.tile([128, COLS], DT.float32)
            ce = pp.tile([128, COLS], DT.float32)
            u2 = pp.tile([128, COLS], DT.float32)
            fo = pp.tile([128, COLS], DT.float32)
            # y = x * (1 - 2t)
            nc.vector.tensor_scalar(
                out=y[:], in0=tgt[:], scalar1=-2.0, scalar2=1.0, op0=OP.mult, op1=OP.add
            )
            nc.vector.tensor_tensor(out=y[:], in0=y[:], in1=lgt[:], op=OP.mult)
            nc.scalar.activation(ce[:], y[:], F.Exp)                  # e^y
            nc.scalar.activation(ce[:], ce[:], F.Ln, bias=1.0)        # ce = ln(1+e^y)
            nc.scalar.activation(u2[:], y[:], F.Exp, scale=-1.0)      # e^-y
            nc.scalar.activation(u2[:], u2[:], F.Ln, bias=1.0)        # ln(1+e^-y)
            nc.scalar.activation(u2[:], u2[:], F.Exp, scale=-2.0)     # u^2
            # fo = ce*u2 ; sums[:,0] = sum(fo)   (TTR unsupported by this
            # walrus build -> TT + reduce)
            nc.vector.tensor_tensor(out=fo[:], in0=ce[:], in1=u2[:], op=OP.mult)
            nc.vector.tensor_reduce(
                out=sums[:, 0:1], in_=fo[:], axis=mybir.AxisListType.X, op=OP.add
            )

            # --- distance grid: per-sample matmuls + min over corners
            minsq = pp.tile([128, COLS], DT.float32)
            for s in range(S):
                lt = wp.tile([K, N], DT.float16, tag="lhsT")
                nc.sync.dma_start(out=lt[:], in_=lhsT[s])
                for b in range(CH // 4):
                    pt = psp.tile([128, 512], DT.float32)
                    for q in range(4):
                        j = b * 4 + q
                        nc.tensor.matmul(
                            out=pt[:, q * 128:(q + 1) * 128],
                            lhsT=lt[:, j * 128:(j + 1) * 128],
                            rhs=rt[:, s * 128:(s + 1) * 128],
                            start=True, stop=True,
                        )
                    c0 = s * CH + b * 4
                    nc.vector.tensor_reduce(
                        out=minsq[:, c0:c0 + 4],
                        in_=pt[:].rearrange("p (c m) -> p c m", m=128),
                        axis=mybir.AxisListType.X,
                        op=OP.min,
                    )

            # --- epilogue: w = exp(-10*sqrt(max(minsq,1e-12))), S2 = sum(fo*w)
            nc.vector.tensor_scalar_max(out=minsq[:], in0=minsq[:], scalar1=1e-12)
            nc.scalar.activation(minsq[:], minsq[:], F.Ln)
            nc.scalar.activation(minsq[:], minsq[:], F.Exp, scale=0.5)   # sqrt
            nc.scalar.activation(minsq[:], minsq[:], F.Exp, scale=-10.0)  # w
            nc.vector.tensor_tensor(out=y[:], in0=fo[:], in1=minsq[:], op=OP.mult)
            nc.vector.tensor_reduce(
                out=sums[:, 1:2], in_=y[:], axis=mybir.AxisListType.X, op=OP.add
            )
            nc.sync.dma_start(out=out[:], in_=sums[:])

    from waitsplit import split_waits

    split_waits(nc)
    return nc


def pack_inputs(inputs, targets, point_coords, corner_coords):
    """Host-side shard + layout packing. Returns in_maps for 8 cores."""
    x = np.asarray(inputs, np.float32)
    t = np.asarray(targets, np.float32)
    pc = np.asarray(point_coords, np.float32)
    cc = np.asarray(corner_coords, np.float32)

    pts = pc[..., :3]                                   # [B,N,3]
    q = (pts * pts).sum(-1)                             # [B,N]
    lhsT = np.empty((B, K, N), np.float16)
    lhsT[:, 0] = pts[..., 0]
    lhsT[:, 1] = pts[..., 1]
    lhsT[:, 2] = pts[..., 2]
    lhsT[:, 3] = q
    lhsT[:, 4] = 1.0

    valid = cc[..., 0] > -1.0                           # [B,M]
    c2p = (cc * cc).sum(-1) + PEN * (~valid)            # [B,M]
    rhs = np.empty((B, K, M), np.float16)
    rhs[:, 0] = -2.0 * cc[..., 0]
    rhs[:, 1] = -2.0 * cc[..., 1]
    rhs[:, 2] = -2.0 * cc[..., 2]
    rhs[:, 3] = 1.0
    rhs[:, 4] = c2p

    in_maps = []
    for c in range(NCORES):
        sl = slice(c * S, (c + 1) * S)
        # [S,N] -> [S, CH, 128pt] -> [128pt, S, CH] -> [128, COLS]
        lgp = x[sl].reshape(S, CH, 128).transpose(2, 0, 1).reshape(128, COLS).copy()
        tgp = t[sl].reshape(S, CH, 128).transpose(2, 0, 1).reshape(128, COLS).copy()
        rhp = rhs[sl].transpose(1, 0, 2).reshape(K, S * 128).copy()  # [K, S*128]
        in_maps.append({
            "lhsT": np.ascontiguousarray(lhsT[sl]),
            "rhs": rhp,
            "lg": lgp,
            "tg": tgp,
        })
    return in_maps


def _finalize(results):
    s1 = 0.0
    s2 = 0.0
    for r in results:
        o = np.asarray(r["out"], np.float64)
        s1 += o[:, 0].sum()
        s2 += o[:, 1].sum()
    bn = float(B * N)
    focal = s1 / bn
    distance = (s1 + 2.0 * s2) / bn
    total = focal + distance
    return (
        np.float32(total),
        np.float32(focal),
        np.float32(distance),
    )


def kernel(inputs, targets, point_coords, corner_coords):
    if "nc" not in _CACHE:
        _CACHE["nc"] = build_nc()
    nc = _CACHE["nc"]
    in_maps = pack_inputs(inputs, targets, point_coords, corner_coords)
    res = run_bass_kernel_spmd(nc, in_maps, core_ids=list(range(NCORES)))
    return _finalize(res.results)


if __name__ == "__main__":
    rng = np.random.default_rng(0)
    ins = {
        "inputs": rng.standard_normal((B, N), dtype=np.float32),
        "targets": (rng.random((B, N)) < 0.05).astype(np.float32),
        "point_coords": rng.random((B, N, 6), dtype=np.float32),
        "corner_coords": rng.random((B, 128, 3), dtype=np.float32),
    }
    print(kernel(**ins))


# revision 4
# speedup vs baseline: 1.3647x; 1.3647x over previous
"""AdaptiveCornerLoss on 8 TRN2 NeuronCores — batch-parallel Bass/Tile kernel.

Shapes (hardcoded): B=64, N=16384, C=6, M=128 corners. 8 cores, 8 samples/core.

Math:
  focal    = u^2 * ce  with  y=(1-2t)*x, ce=softplus(y)=ln(1+e^y),
             u=sigmoid(y)  =>  u^2 = exp(-2*ln(1+e^{-y}))
  d2(n,m)  = |p|^2 + |c|^2 - 2 p.c   (augmented fp16 matmul; per-point feature
             rows [px,py,pz,|p|^2,1] vs corner rows [-2cx,-2cy,-2cz,1,|c|^2+pen])
  w        = exp(-10*sqrt(max(min_m d2, 1e-12))) via exp/ln only (one ACT set)

Layout tricks:
  * Valid corners are host-compacted per sample; kernel is built for
    Mk = roundup(max valid count, 32) corners (96 for the graded data).
    Padding corners get |c|^2+pen large so they never win the min; a sample
    with zero valid corners still matches the reference (w -> exp(-large)=0).
  * CPG=4 chunks share one LDWEIGHTS: stationary K=20 stacks 4 chunks'
    features; 4 rhs variants zero all but their 5 feature rows.
  * PSUM tiles span 4 banks; each bank holds 4 chunks' [128, Mk] grids; one
    strided tensor_reduce(min) consumes 16 chunks.
Outputs per core: per-partition partial sums [128,2] of (focal, focal*w);
host reduces and forms (total, focal_loss, distance_loss).
"""

import sys

sys.path.insert(0, "/opt/trn_rl_repo")
sys.path.insert(0, "/root/problem")

import numpy as np

import concourse.bass as bass
import concourse.mybir as mybir
from concourse import tile
from concourse.bass_utils import run_bass_kernel_spmd
from waitsplit import split_waits

NCORES = 8
B, N, M = 64, 16384, 128
S = B // NCORES          # samples per core
K = 5                    # feature rows per chunk
CPG = 4                  # chunks sharing one LDWEIGHTS group
CH = N // 128            # 128-point chunks per sample (128)
GRP = CH // CPG          # LDW groups per sample (32)
TPS = 8                  # psum tiles per sample (16 chunks each)
COLS = S * CH            # minsq/logit columns per core (1024)
PEN = 100.0

F = mybir.ActivationFunctionType
OP = mybir.AluOpType
DT = mybir.dt

_CACHE = {}


def build_nc(Mk):
    nc = bass.Bass()
    lhsT = nc.declare_dram_parameter(
        "lhsT", [S, K * CPG, N // CPG], DT.float16, isOutput=False
    )
    rhs = nc.declare_dram_parameter(
        "rhs", [K * CPG, S * CPG * Mk], DT.float16, isOutput=False
    )
    lg = nc.declare_dram_parameter("lg", [128, COLS], DT.float32, isOutput=False)
    tg = nc.declare_dram_parameter("tg", [128, COLS], DT.float32, isOutput=False)
    out = nc.declare_dram_parameter("out", [128, 2], DT.float32, isOutput=True)

    with tile.TileContext(nc) as tc:
        with (
            tc.tile_pool(name="persist", bufs=1) as pp,
            tc.tile_pool(name="stream", bufs=2) as wp,
            tc.tile_pool(name="psum", bufs=2, space="PSUM") as psp,
        ):
            # --- resident inputs
            rt = pp.tile([K * CPG, S * CPG * Mk], DT.float16)
            nc.sync.dma_start(out=rt[:], in_=rhs[:])
            lgt = pp.tile([128, COLS], DT.float32)
            nc.sync.dma_start(out=lgt[:], in_=lg[:])
            tgt = pp.tile([128, COLS], DT.float32)
            nc.sync.dma_start(out=tgt[:], in_=tg[:])

            sums = pp.tile([128, 2], DT.float32)

            # --- focal chain (mostly ACT; overlaps the grid work below)
            y = pp.tile([128, COLS], DT.float32)
            ce = pp.tile([128, COLS], DT.float32)
            u2 = pp.tile([128, COLS], DT.float32)
            fo = pp.tile([128, COLS], DT.float32)
            nc.vector.tensor_scalar(
                out=y[:], in0=tgt[:], scalar1=-2.0, scalar2=1.0, op0=OP.mult, op1=OP.add
            )
            nc.vector.tensor_tensor(out=y[:], in0=y[:], in1=lgt[:], op=OP.mult)
            nc.scalar.activation(ce[:], y[:], F.Exp)                  # e^y
            nc.scalar.activation(ce[:], ce[:], F.Ln, bias=1.0)        # ce = ln(1+e^y)
            nc.scalar.activation(u2[:], y[:], F.Exp, scale=-1.0)      # e^-y
            nc.scalar.activation(u2[:], u2[:], F.Ln, bias=1.0)        # ln(1+e^-y)
            nc.scalar.activation(u2[:], u2[:], F.Exp, scale=-2.0)     # u^2
            nc.vector.tensor_tensor(out=fo[:], in0=ce[:], in1=u2[:], op=OP.mult)
            nc.vector.tensor_reduce(
                out=sums[:, 0:1], in_=fo[:], axis=mybir.AxisListType.X, op=OP.add
            )

            # --- distance grid
            minsq = pp.tile([128, COLS], DT.float32)
            for s in range(S):
                lt = wp.tile([K * CPG, N // CPG], DT.float16, tag="lhsT")
                nc.sync.dma_start(out=lt[:], in_=lhsT[s])
                for t in range(TPS):
                    pt = psp.tile([128, 2048], DT.float32)  # 4 banks, 16 chunks
                    for b in range(4):                      # bank = one LDW group
                        g = t * 4 + b
                        for v in range(CPG):
                            nc.tensor.matmul(
                                out=pt[:, 512 * b + Mk * v: 512 * b + Mk * (v + 1)],
                                lhsT=lt[:, g * 128:(g + 1) * 128],
                                rhs=rt[:, (s * CPG + v) * Mk:(s * CPG + v + 1) * Mk],
                                start=True, stop=True,
                            )
                    c0 = s * CH + t * 16
                    src = pt[:].rearrange("p (b r) -> p b r", r=512)[:, :, 0: CPG * Mk]
                    src = src.rearrange("p b (v m) -> p b v m", m=Mk)
                    nc.vector.tensor_reduce(
                        out=minsq[:, c0:c0 + 16],
                        in_=src,
                        axis=mybir.AxisListType.X,
                        op=OP.min,
                    )

            # --- epilogue: w = exp(-10*sqrt(max(minsq,1e-12))), S2 = sum(fo*w)
            nc.vector.tensor_scalar_max(out=minsq[:], in0=minsq[:], scalar1=1e-12)
            nc.scalar.activation(minsq[:], minsq[:], F.Ln)
            nc.scalar.activation(minsq[:], minsq[:], F.Exp, scale=0.5)    # sqrt
            nc.scalar.activation(minsq[:], minsq[:], F.Exp, scale=-10.0)  # w
            nc.vector.tensor_tensor(out=y[:], in0=fo[:], in1=minsq[:], op=OP.mult)
            nc.vector.tensor_reduce(
                out=sums[:, 1:2], in_=y[:], axis=mybir.AxisListType.X, op=OP.add
            )
            nc.sync.dma_start(out=out[:], in_=sums[:])

    split_waits(nc)
    return nc


def pack_inputs(inputs, targets, point_coords, corner_coords):
    """Host-side shard + layout packing. Returns (in_maps, Mk)."""
    x = np.asarray(inputs, np.float32)
    t = np.asarray(targets, np.float32)
    pc = np.asarray(point_coords, np.float32)
    cc = np.asarray(corner_coords, np.float32)

    # point features, grouped CPG chunks per LDW: lhsT[s] = [K*CPG, N/CPG],
    # rows v*K+k = feature k of chunk (4g+v) at group-col (g*128+m)
    pts = pc[..., :3]
    q = (pts * pts).sum(-1)
    feats = np.empty((B, K, N), np.float32)
    feats[:, 0] = pts[..., 0]
    feats[:, 1] = pts[..., 1]
    feats[:, 2] = pts[..., 2]
    feats[:, 3] = q
    feats[:, 4] = 1.0
    # [B, K, CH, 128] -> [B, CPG, K, GRP, 128] -> [B, K*CPG, GRP*128]
    fg = feats.reshape(B, K, GRP, CPG, 128).transpose(0, 3, 1, 2, 4)
    lhsT = fg.reshape(B, K * CPG, N // CPG).astype(np.float16)

    # corners: compact valid ones to the front, pad with PEN sentinels
    valid = cc[..., 0] > -1.0                     # [B, M]
    nv = valid.sum(-1)
    maxv = int(nv.max()) if nv.max() > 0 else 1
    Mk = min(M, ((maxv + 31) // 32) * 32)
    cfeat = np.zeros((B, K, Mk), np.float32)
    cfeat[:, 4] = PEN                              # default: padding corner
    for b in range(B):
        v = cc[b][valid[b]]                        # [nv, 3]
        n = v.shape[0]
        cfeat[b, 0, :n] = -2.0 * v[:, 0]
        cfeat[b, 1, :n] = -2.0 * v[:, 1]
        cfeat[b, 2, :n] = -2.0 * v[:, 2]
        cfeat[b, 3, :n] = 1.0
        cfeat[b, 4, :n] = (v * v).sum(-1)
    # rhs variants: [B, CPG, K*CPG, Mk], variant v has rows v*K..v*K+K live
    rhs = np.zeros((B, CPG, K * CPG, Mk), np.float32)
    for v in range(CPG):
        rhs[:, v, v * K:(v + 1) * K, :] = cfeat
    rhs = rhs.astype(np.float16)

    in_maps = []
    for c in range(NCORES):
        sl = slice(c * S, (c + 1) * S)
        lgp = x[sl].reshape(S, CH, 128).transpose(2, 0, 1).reshape(128, COLS).copy()
        tgp = t[sl].reshape(S, CH, 128).transpose(2, 0, 1).reshape(128, COLS).copy()
        # [S, CPG, K*CPG, Mk] -> [K*CPG, S*CPG*Mk]
        rhp = rhs[sl].transpose(2, 0, 1, 3).reshape(K * CPG, S * CPG * Mk).copy()
        in_maps.append({
            "lhsT": np.ascontiguousarray(lhsT[sl]),
            "rhs": rhp,
            "lg": lgp,
            "tg": tgp,
        })
    return in_maps, Mk


def _finalize(results):
    s1 = 0.0
    s2 = 0.0
    for r in results:
        o = np.asarray(r["out"], np.float64)
        s1 += o[:, 0].sum()
        s2 += o[:, 1].sum()
    bn = float(B * N)
    focal = s1 / bn
    distance = (s1 + 2.0 * s2) / bn
    total = focal + distance
    return (np.float32(total), np.float32(focal), np.float32(distance))


def kernel(inputs, targets, point_coords, corner_coords):
    in_maps, Mk = pack_inputs(inputs, targets, point_coords, corner_coords)
    if Mk not in _CACHE:
        _CACHE[Mk] = build_nc(Mk)
    nc = _CACHE[Mk]
    res = run_bass_kernel_spmd(nc, in_maps, core_ids=list(range(NCORES)))
    return _finalize(res.results)


if __name__ == "__main__":
    rng = np.random.default_rng(0)
    ins = {
        "inputs": rng.standard_normal((B, N), dtype=np.float32),
        "targets": (rng.random((B, N)) < 0.05).astype(np.float32),
        "point_coords": rng.random((B, N, 6), dtype=np.float32),
        "corner_coords": rng.random((B, 128, 3), dtype=np.float32),
    }
    print(kernel(**ins))
